# revision 2
# baseline (speedup 1.0000x reference)
"""Trainium2 Bass kernel for nn_Blur: 4x4 FIR depthwise blur with pad (2,1).

out[n,c,i,j] = sum_{a,b} K[a,b] * x[n,c, i+1-a, j+1-b]   (zero-padded)

Strategy (8 NeuronCores, pure data parallelism over the 8192 (n,c) slices,
bf16 I/O to halve HBM traffic — the 2e-2 gate leaves ~5x margin):
  - Each core processes 1024 slices of 64x64, 16 SBUF tiles of 64 slices.
  - W-parity packing: partition p = wp*64 + u (wp = w%2, u = h), free =
    (slice, gc) with gc = 1 + w//2 plus one zero pad column on each side.
    The 4 W-taps of an output column then span only THREE gc columns, so
    the whole 4x4 blur is 3 PSUM-accumulated matmuls (vs 4 for the
    member-packed layout): lhsT_sh[(wp,u),(jp,i)] = K[i+1-u, jp-wp+1-2sh].
  - bf16 matmuls run at 1 col/cycle; weights (1/16, 3/16, 9/16 scale) are
    exact in bf16, accumulation is fp32 in PSUM.
  - PSUM->SBUF copies cast fp32->bf16 and alternate DVE/ACT engines.
  - The host pre-permutes each core's shard into the exact SBUF tile
    layout, so every DMA descriptor is one contiguous run per partition.
"""

import sys
import types

import numpy as np
import ml_dtypes

import concourse.bacc as bacc
import concourse.mybir as mybir
from concourse.tile import TileContext
from concourse.bass_utils import run_bass_kernel_spmd

BF16_NP = ml_dtypes.bfloat16


def _install_ntff_hook():
    """Best-effort shim: this image's antenv lacks axon_hooks, which the
    trace=True path of run_bass_kernel_spmd imports. Harmless if unused."""
    if "antenv.axon_hooks" in sys.modules:
        return
    try:
        sys.path.insert(0, "/root/.axon_site")
        from trn_agent_boot.trn_boot import _ntff_profile_via_ctypes

        hook = _ntff_profile_via_ctypes("/opt/axon/libaxon_pjrt.so")
        mod = types.ModuleType("antenv.axon_hooks")
        mod.get_axon_ntff_profile_hook = lambda: hook
        mod.set_axon_ntff_profile_hook = lambda h: None
        sys.modules["antenv.axon_hooks"] = mod
    except Exception:
        pass


_install_ntff_hook()

N_CORES = 8
B, C, H, W = 32, 256, 64, 64
NSLICES = B * C                       # 8192
SLICES_PER_CORE = NSLICES // N_CORES  # 1024
TILE_SLICES = 64                      # slices per SBUF tile
G = W // 2                            # 32 w-parity column groups
GC = G + 2                            # + zero pad col on each side
QS = 16                               # slices per PSUM group (16*32 = 512)
F32 = mybir.dt.float32
BF16 = mybir.dt.bfloat16

_NC_CACHE = {}


def _build_wmat(K: np.ndarray) -> np.ndarray:
    """(4, 128, 128) bf16: per-shift stationary matrices (4th = warmup pad).

    lhsT_sh[(wp,u), (jp,i)] = K[a, b] with a = i+1-u, b = jp-wp+1-2*sh,
    for shifts sh in (-1, 0, +1); entries with a or b outside 0..3 are 0.
    """
    K = np.asarray(K, np.float32)
    wmat = np.zeros((4, 128, 128), np.float32)
    for si, sh in enumerate((-1, 0, 1)):
        for wp in range(2):
            for jp in range(2):
                b = jp - wp + 1 - 2 * sh
                if not 0 <= b <= 3:
                    continue
                T = np.zeros((H, H), np.float32)
                for i in range(H):
                    for u in range(max(0, i - 2), min(H, i + 2)):
                        T[u, i] = K[i + 1 - u, b]
                wmat[si, wp * 64 : wp * 64 + 64, jp * 64 : jp * 64 + 64] = T
    return wmat.astype(BF16_NP)


WARMUP_MMS = 12


def _build_nc(slices_per_core: int = SLICES_PER_CORE):
    ntiles = slices_per_core // TILE_SLICES
    nc = bacc.Bacc("TRN2", target_bir_lowering=False, debug=False)
    # DRAM layouts are the SBUF tile layouts (host pre-/post-permutes):
    #   x: [tile, p=(wp u), (s gc)] with gc zero-padded to 34 — bf16
    #   y: [tile, p=(jp i), (s g)] — bf16
    x = nc.dram_tensor(
        "x", [ntiles, 128, TILE_SLICES * GC], BF16, kind="ExternalInput"
    ).ap()
    wm = nc.dram_tensor("w", [4, 128, 128], BF16, kind="ExternalInput").ap()
    y = nc.dram_tensor(
        "y", [ntiles, 128, TILE_SLICES * G], BF16, kind="ExternalOutput"
    ).ap()
    # sink for the PE warm-up matmuls (kept alive so DCE can't drop them)
    warm_out = nc.dram_tensor("warm", [128, 4], F32, kind="ExternalOutput").ap()

    with TileContext(nc) as tc:
        with (
            tc.tile_pool(name="wpool", bufs=1) as wpool,
            tc.tile_pool(name="xpool", bufs=6) as xpool,
            tc.tile_pool(name="opool", bufs=8) as opool,
            tc.tile_pool(name="pspool", bufs=8, space="PSUM") as pspool,
        ):
            wsb = wpool.tile([128, 4, 128], BF16, name="wsb")
            nc.sync.dma_start(wsb[:], wm.rearrange("b k m -> k b m"))

            # PE warm-up: matmuls on the weight tile while the first input
            # tiles stream in, so the HAM clock gate opens (1.2 -> 2.4 GHz)
            # before the real matmuls start. Only depends on wsb.
            wscratch = wpool.tile([128, 4], F32, name="wscratch")
            wps = pspool.tile([128, 512], F32, name="wps", tag="ps")
            for r in range(WARMUP_MMS):
                nc.tensor.matmul(
                    wps[:],
                    wsb[:, 0, :],
                    wsb.rearrange("p b m -> p (b m)"),
                    start=(r == 0),
                    stop=(r == WARMUP_MMS - 1),
                )
            nc.vector.tensor_copy(wscratch[:], wps[:, 0:4])
            nc.sync.dma_start(warm_out, wscratch[:])

            for t in range(ntiles):
                xt = xpool.tile([128, TILE_SLICES, GC], BF16, name="xt")
                nc.sync.dma_start(xt[:], x[t])

                for q in range(TILE_SLICES // QS):
                    ps = pspool.tile([128, QS, G], F32, name="ps")
                    for si in range(3):
                        nc.tensor.matmul(
                            ps[:],
                            wsb[:, si, :],
                            xt[:, QS * q : QS * q + QS, si : si + G],
                            start=(si == 0),
                            stop=(si == 2),
                        )
                    oq = opool.tile([128, QS * G], BF16, name="oq")
                    # alternate copy engine: DVE and ACT share the load
                    if q % 2 == 0:
                        nc.vector.tensor_copy(oq[:], ps[:])
                    else:
                        nc.scalar.copy(oq[:], ps[:])

                    # store each psum-group's slab as soon as its copy lands
                    # (ACT HWDGE ring): the store stream starts early and
                    # never head-of-line-blocks the SP ring feeding loads.
                    # The last tile's stores split across BOTH rings.
                    last_tile = t == ntiles - 1
                    store_eng = nc.sync if (last_tile and q % 2) else nc.scalar
                    store_eng.dma_start(
                        y[t][:, QS * G * q : QS * G * (q + 1)], oq[:]
                    )

    nc.compile()
    return nc


def get_nc(slices_per_core: int = SLICES_PER_CORE):
    if slices_per_core not in _NC_CACHE:
        _NC_CACHE[slices_per_core] = _build_nc(slices_per_core)
    return _NC_CACHE[slices_per_core]


def _pack_input(xs: np.ndarray) -> np.ndarray:
    """[S, H, W] fp32 -> [S/64, 128, 64*34] bf16 in the SBUF tile layout."""
    s = xs.shape[0]
    ntiles = s // TILE_SLICES
    # (t, s, u, g, wp) -> (t, wp, u, s, g)
    v = xs.reshape(ntiles, TILE_SLICES, H, G, 2).transpose(0, 4, 2, 1, 3)
    out = np.zeros((ntiles, 2, H, TILE_SLICES, GC), BF16_NP)
    out[:, :, :, :, 1 : 1 + G] = v.astype(BF16_NP)
    return np.ascontiguousarray(out.reshape(ntiles, 128, TILE_SLICES * GC))


def _unpack_output(yp: np.ndarray) -> np.ndarray:
    """[S/64, 128, 64*32] bf16 -> [S, H, W] fp32."""
    ntiles = yp.shape[0]
    # [(jp, i), (s, g)] -> [s, i, (g, jp)]
    v = yp.reshape(ntiles, 2, H, TILE_SLICES, G).transpose(0, 3, 2, 4, 1)
    return v.reshape(ntiles * TILE_SLICES, H, W).astype(np.float32)


def kernel(x: np.ndarray, kernel: np.ndarray, _trace: bool = False, **_tkw):
    x = np.asarray(x, np.float32)
    wmat = _build_wmat(kernel)
    b, c, h, w = x.shape
    xs = x.reshape(b * c, h, w)
    spc = (b * c) // N_CORES
    nc = get_nc(spc)
    in_maps = [
        {"x": _pack_input(xs[k * spc : (k + 1) * spc]), "w": wmat}
        for k in range(N_CORES)
    ]
    res = run_bass_kernel_spmd(
        nc, in_maps, list(range(N_CORES)), trace=_trace, **_tkw
    )
    out = np.concatenate(
        [_unpack_output(res.results[k]["y"]) for k in range(N_CORES)], axis=0
    )
    result = out.reshape(b, c, h, w)
    if _trace:
        return result, res
    return result


# revision 4
# speedup vs baseline: 1.1544x; 1.1544x over previous
"""Trainium2 Bass kernel for nn_Blur: 4x4 FIR depthwise blur with pad (2,1).

out[n,c,i,j] = sum_{a,b} K[a,b] * x[n,c, i+1-a, j+1-b]   (zero-padded)

Strategy (8 NeuronCores, pure data parallelism over the 8192 (n,c) slices,
bf16 I/O to halve HBM traffic — the 2e-2 gate leaves ~5x margin):
  - Each core processes 1024 slices of 64x64, 16 SBUF tiles of 64 slices.
  - W-parity packing: partition p = wp*64 + u (wp = w%2, u = h), free =
    (slice, gc) with gc = 1 + w//2 plus one zero pad column on each side.
    The 4 W-taps of an output column then span only THREE gc columns, so
    the whole 4x4 blur is 3 PSUM-accumulated matmuls (vs 4 for the
    member-packed layout): lhsT_sh[(wp,u),(jp,i)] = K[i+1-u, jp-wp+1-2sh].
  - bf16 matmuls run at 1 col/cycle; weights (1/16, 3/16, 9/16 scale) are
    exact in bf16, accumulation is fp32 in PSUM.
  - PSUM->SBUF copies cast fp32->bf16 and alternate DVE/ACT engines.
  - The host pre-permutes each core's shard into the exact SBUF tile
    layout, so every DMA descriptor is one contiguous run per partition.
"""

import sys
import types

import numpy as np
import ml_dtypes

import concourse.bacc as bacc
import concourse.mybir as mybir
from concourse.tile import TileContext
from concourse.bass_utils import run_bass_kernel_spmd

BF16_NP = ml_dtypes.bfloat16


def _install_ntff_hook():
    """Best-effort shim: this image's antenv lacks axon_hooks, which the
    trace=True path of run_bass_kernel_spmd imports. Harmless if unused."""
    if "antenv.axon_hooks" in sys.modules:
        return
    try:
        sys.path.insert(0, "/root/.axon_site")
        from trn_agent_boot.trn_boot import _ntff_profile_via_ctypes

        hook = _ntff_profile_via_ctypes("/opt/axon/libaxon_pjrt.so")
        mod = types.ModuleType("antenv.axon_hooks")
        mod.get_axon_ntff_profile_hook = lambda: hook
        mod.set_axon_ntff_profile_hook = lambda h: None
        sys.modules["antenv.axon_hooks"] = mod
    except Exception:
        pass


_install_ntff_hook()

N_CORES = 8
B, C, H, W = 32, 256, 64, 64
NSLICES = B * C                       # 8192
SLICES_PER_CORE = NSLICES // N_CORES  # 1024
TILE_SLICES = 64                      # slices per SBUF tile
G = W // 2                            # 32 w-parity column groups
GC = G + 2                            # + zero pad col on each side
QS = 16                               # slices per PSUM group (16*32 = 512)
F32 = mybir.dt.float32
BF16 = mybir.dt.bfloat16

_NC_CACHE = {}


def _build_wmat(K: np.ndarray) -> np.ndarray:
    """(4, 128, 128) bf16: per-shift stationary matrices (4th = warmup pad).

    lhsT_sh[(wp,u), (jp,i)] = K[a, b] with a = i+1-u, b = jp-wp+1-2*sh,
    for shifts sh in (-1, 0, +1); entries with a or b outside 0..3 are 0.
    """
    K = np.asarray(K, np.float32)
    wmat = np.zeros((4, 128, 128), np.float32)
    for si, sh in enumerate((-1, 0, 1)):
        for wp in range(2):
            for jp in range(2):
                b = jp - wp + 1 - 2 * sh
                if not 0 <= b <= 3:
                    continue
                T = np.zeros((H, H), np.float32)
                for i in range(H):
                    for u in range(max(0, i - 2), min(H, i + 2)):
                        T[u, i] = K[i + 1 - u, b]
                wmat[si, wp * 64 : wp * 64 + 64, jp * 64 : jp * 64 + 64] = T
    return wmat.astype(BF16_NP)


WARMUP_MMS = 12


def _build_nc(slices_per_core: int = SLICES_PER_CORE):
    ntiles = slices_per_core // TILE_SLICES
    nc = bacc.Bacc("TRN2", target_bir_lowering=False, debug=False)
    # DRAM layouts are the SBUF tile layouts (host pre-/post-permutes):
    #   x: [tile, p=(wp u), (s gc)] with gc zero-padded to 34 — bf16
    #   y: [tile, p=(jp i), (s g)] — bf16
    x = nc.dram_tensor(
        "x", [ntiles, 128, TILE_SLICES * GC], BF16, kind="ExternalInput"
    ).ap()
    wm = nc.dram_tensor("w", [4, 128, 128], BF16, kind="ExternalInput").ap()
    y = nc.dram_tensor(
        "y", [ntiles, 128, TILE_SLICES * G], BF16, kind="ExternalOutput"
    ).ap()
    # sink for the PE warm-up matmuls (kept alive so DCE can't drop them)
    warm_out = nc.dram_tensor("warm", [128, 4], F32, kind="ExternalOutput").ap()

    with TileContext(nc) as tc:
        with (
            tc.tile_pool(name="wpool", bufs=1) as wpool,
            tc.tile_pool(name="xpool", bufs=8) as xpool,
            tc.tile_pool(name="opool", bufs=4) as opool,
            tc.tile_pool(name="pspool", bufs=8, space="PSUM") as pspool,
        ):
            wsb = wpool.tile([128, 4, 128], BF16, name="wsb")
            nc.sync.dma_start(wsb[:], wm.rearrange("b k m -> k b m"))

            # PE warm-up: matmuls on the weight tile while the first input
            # tiles stream in, so the HAM clock gate opens (1.2 -> 2.4 GHz)
            # before the real matmuls start. Only depends on wsb.
            wscratch = wpool.tile([128, 4], F32, name="wscratch")
            wps = pspool.tile([128, 512], F32, name="wps", tag="ps")
            for r in range(WARMUP_MMS):
                nc.tensor.matmul(
                    wps[:],
                    wsb[:, 0, :],
                    wsb.rearrange("p b m -> p (b m)"),
                    start=(r == 0),
                    stop=(r == WARMUP_MMS - 1),
                )
            nc.vector.tensor_copy(wscratch[:], wps[:, 0:4])
            nc.sync.dma_start(warm_out, wscratch[:])

            for t in range(ntiles):
                xt = xpool.tile([128, TILE_SLICES, GC], BF16, name="xt")
                nc.sync.dma_start(xt[:], x[t])

                # one output tile per input tile; 4 psum-group copies fill it
                ot = opool.tile([128, TILE_SLICES, G], BF16, name="ot")
                for q in range(TILE_SLICES // QS):
                    ps = pspool.tile([128, QS, G], F32, name="ps")
                    for si in range(3):
                        nc.tensor.matmul(
                            ps[:],
                            wsb[:, si, :],
                            xt[:, QS * q : QS * q + QS, si : si + G],
                            start=(si == 0),
                            stop=(si == 2),
                        )
                    # alternate copy engine: DVE and ACT share the load.
                    # DMA issues all live on the SP ring, so neither copy
                    # engine's sequencer ever stalls behind a 600ns
                    # DIRECT2D descriptor write.
                    if q % 2 == 0:
                        nc.vector.tensor_copy(
                            ot[:, QS * q : QS * q + QS, :], ps[:]
                        )
                    else:
                        nc.scalar.copy(ot[:, QS * q : QS * q + QS, :], ps[:])

                # single whole-tile store: 16 store issues total instead of
                # 64, keeping sequencer descriptor-write cost off the
                # per-psum-group critical path
                nc.sync.dma_start(y[t], ot[:])

    nc.compile()
    return nc


def get_nc(slices_per_core: int = SLICES_PER_CORE):
    if slices_per_core not in _NC_CACHE:
        _NC_CACHE[slices_per_core] = _build_nc(slices_per_core)
    return _NC_CACHE[slices_per_core]


def _pack_input(xs: np.ndarray) -> np.ndarray:
    """[S, H, W] fp32 -> [S/64, 128, 64*34] bf16 in the SBUF tile layout."""
    s = xs.shape[0]
    ntiles = s // TILE_SLICES
    # (t, s, u, g, wp) -> (t, wp, u, s, g)
    v = xs.reshape(ntiles, TILE_SLICES, H, G, 2).transpose(0, 4, 2, 1, 3)
    out = np.zeros((ntiles, 2, H, TILE_SLICES, GC), BF16_NP)
    out[:, :, :, :, 1 : 1 + G] = v.astype(BF16_NP)
    return np.ascontiguousarray(out.reshape(ntiles, 128, TILE_SLICES * GC))


def _unpack_output(yp: np.ndarray) -> np.ndarray:
    """[S/64, 128, 64*32] bf16 -> [S, H, W] fp32."""
    ntiles = yp.shape[0]
    # [(jp, i), (s, g)] -> [s, i, (g, jp)]
    v = yp.reshape(ntiles, 2, H, TILE_SLICES, G).transpose(0, 3, 2, 4, 1)
    return v.reshape(ntiles * TILE_SLICES, H, W).astype(np.float32)


def kernel(x: np.ndarray, kernel: np.ndarray, _trace: bool = False, **_tkw):
    x = np.asarray(x, np.float32)
    wmat = _build_wmat(kernel)
    b, c, h, w = x.shape
    xs = x.reshape(b * c, h, w)
    spc = (b * c) // N_CORES
    nc = get_nc(spc)
    in_maps = [
        {"x": _pack_input(xs[k * spc : (k + 1) * spc]), "w": wmat}
        for k in range(N_CORES)
    ]
    res = run_bass_kernel_spmd(
        nc, in_maps, list(range(N_CORES)), trace=_trace, **_tkw
    )
    out = np.concatenate(
        [_unpack_output(res.results[k]["y"]) for k in range(N_CORES)], axis=0
    )
    result = out.reshape(b, c, h, w)
    if _trace:
        return result, res
    return result


# revision 8
# speedup vs baseline: 1.1551x; 1.0006x over previous
"""Trainium2 Bass kernel for nn_Blur: 4x4 FIR depthwise blur with pad (2,1).

out[n,c,i,j] = sum_{a,b} K[a,b] * x[n,c, i+1-a, j+1-b]   (zero-padded)

Strategy (8 NeuronCores, pure data parallelism over the 8192 (n,c) slices,
bf16 I/O to halve HBM traffic — the 2e-2 gate leaves ~5x margin):
  - Each core processes 1024 slices of 64x64, 16 SBUF tiles of 64 slices.
  - W-parity packing: partition p = wp*64 + u (wp = w%2, u = h), free =
    (slice, gc) with gc = 1 + w//2 plus one zero pad column on each side.
    The 4 W-taps of an output column then span only THREE gc columns, so
    the whole 4x4 blur is 3 PSUM-accumulated matmuls (vs 4 for the
    member-packed layout): lhsT_sh[(wp,u),(jp,i)] = K[i+1-u, jp-wp+1-2sh].
  - bf16 matmuls run at 1 col/cycle; weights (1/16, 3/16, 9/16 scale) are
    exact in bf16, accumulation is fp32 in PSUM.
  - PSUM->SBUF copies cast fp32->bf16 and alternate DVE/ACT engines.
  - The host pre-permutes each core's shard into the exact SBUF tile
    layout, so every DMA descriptor is one contiguous run per partition.
"""

import sys
import types

import numpy as np
import ml_dtypes

import concourse.bacc as bacc
import concourse.mybir as mybir
from concourse.tile import TileContext
from concourse.bass_utils import run_bass_kernel_spmd

BF16_NP = ml_dtypes.bfloat16


def _install_ntff_hook():
    """Best-effort shim: this image's antenv lacks axon_hooks, which the
    trace=True path of run_bass_kernel_spmd imports. Harmless if unused."""
    if "antenv.axon_hooks" in sys.modules:
        return
    try:
        sys.path.insert(0, "/root/.axon_site")
        from trn_agent_boot.trn_boot import _ntff_profile_via_ctypes

        hook = _ntff_profile_via_ctypes("/opt/axon/libaxon_pjrt.so")
        mod = types.ModuleType("antenv.axon_hooks")
        mod.get_axon_ntff_profile_hook = lambda: hook
        mod.set_axon_ntff_profile_hook = lambda h: None
        sys.modules["antenv.axon_hooks"] = mod
    except Exception:
        pass


_install_ntff_hook()

N_CORES = 8
B, C, H, W = 32, 256, 64, 64
NSLICES = B * C                       # 8192
SLICES_PER_CORE = NSLICES // N_CORES  # 1024
TILE_SLICES = 64                      # slices per SBUF tile
G = W // 2                            # 32 w-parity column groups
GC = G + 2                            # + zero pad col on each side
QS = 16                               # slices per PSUM group (16*32 = 512)
F32 = mybir.dt.float32
BF16 = mybir.dt.bfloat16

_NC_CACHE = {}


def _build_wmat(K: np.ndarray) -> np.ndarray:
    """(4, 128, 128) bf16: per-shift stationary matrices (4th = warmup pad).

    lhsT_sh[(wp,u), (jp,i)] = K[a, b] with a = i+1-u, b = jp-wp+1-2*sh,
    for shifts sh in (-1, 0, +1); entries with a or b outside 0..3 are 0.
    """
    K = np.asarray(K, np.float32)
    wmat = np.zeros((4, 128, 128), np.float32)
    for si, sh in enumerate((-1, 0, 1)):
        for wp in range(2):
            for jp in range(2):
                b = jp - wp + 1 - 2 * sh
                if not 0 <= b <= 3:
                    continue
                T = np.zeros((H, H), np.float32)
                for i in range(H):
                    for u in range(max(0, i - 2), min(H, i + 2)):
                        T[u, i] = K[i + 1 - u, b]
                wmat[si, wp * 64 : wp * 64 + 64, jp * 64 : jp * 64 + 64] = T
    return wmat.astype(BF16_NP)


WARMUP_MMS = 7


def _build_nc(slices_per_core: int = SLICES_PER_CORE):
    ntiles = slices_per_core // TILE_SLICES
    nc = bacc.Bacc("TRN2", target_bir_lowering=False, debug=False)
    # DRAM layouts are the SBUF tile layouts (host pre-/post-permutes):
    #   x: [tile, p=(wp u), (s gc)] with gc zero-padded to 34 — bf16
    #   y: [tile, p=(jp i), (s g)] — bf16
    x = nc.dram_tensor(
        "x", [ntiles, 128, TILE_SLICES * GC], BF16, kind="ExternalInput"
    ).ap()
    wm = nc.dram_tensor("w", [4, 128, 128], BF16, kind="ExternalInput").ap()
    y = nc.dram_tensor(
        "y", [ntiles, 128, TILE_SLICES * G], BF16, kind="ExternalOutput"
    ).ap()
    # sink for the PE warm-up matmuls (kept alive so DCE can't drop them)
    warm_out = nc.dram_tensor("warm", [128, 4], F32, kind="ExternalOutput").ap()

    with TileContext(nc) as tc:
        with (
            tc.tile_pool(name="wpool", bufs=1) as wpool,
            tc.tile_pool(name="xpool", bufs=8) as xpool,
            tc.tile_pool(name="opool", bufs=4) as opool,
            tc.tile_pool(name="pspool", bufs=8, space="PSUM") as pspool,
        ):
            # weights + warmup sink ride the ACT ring; the SP ring issues
            # ONLY the input-tile loads so prefetch is never head-of-line
            # blocked behind a store's semaphore wait (in-order sequencer)
            wsb = wpool.tile([128, 4, 128], BF16, name="wsb")
            nc.scalar.dma_start(wsb[:], wm.rearrange("b k m -> k b m"))

            # PE warm-up: matmuls on the weight tile while the first input
            # tiles stream in, so the HAM clock gate opens (1.2 -> 2.4 GHz)
            # before the real matmuls start. Only depends on wsb.
            wscratch = wpool.tile([128, 4], F32, name="wscratch")
            wps = pspool.tile([128, 512], F32, name="wps", tag="ps")
            for r in range(WARMUP_MMS):
                nc.tensor.matmul(
                    wps[:],
                    wsb[:, 0, :],
                    wsb.rearrange("p b m -> p (b m)"),
                    start=(r == 0),
                    stop=(r == WARMUP_MMS - 1),
                )
            nc.vector.tensor_copy(wscratch[:], wps[:, 0:4])
            nc.scalar.dma_start(warm_out, wscratch[:])

            for t in range(ntiles):
                xt = xpool.tile([128, TILE_SLICES, GC], BF16, name="xt")
                nc.sync.dma_start(xt[:], x[t])

                # one output tile per input tile; 4 psum-group copies fill it
                ot = opool.tile([128, TILE_SLICES, G], BF16, name="ot")
                for q in range(TILE_SLICES // QS):
                    ps = pspool.tile([128, QS, G], F32, name="ps")
                    for si in range(3):
                        nc.tensor.matmul(
                            ps[:],
                            wsb[:, si, :],
                            xt[:, QS * q : QS * q + QS, si : si + G],
                            start=(si == 0),
                            stop=(si == 2),
                        )
                    # alternate copy engine: DVE and ACT share the load.
                    # DMA issues all live on the SP ring, so neither copy
                    # engine's sequencer ever stalls behind a 600ns
                    # DIRECT2D descriptor write.
                    if q % 2 == 0:
                        nc.vector.tensor_copy(
                            ot[:, QS * q : QS * q + QS, :], ps[:]
                        )
                    else:
                        nc.scalar.copy(ot[:, QS * q : QS * q + QS, :], ps[:])

                # single whole-tile store: 16 store issues total instead of
                # 64, keeping sequencer descriptor-write cost off the
                # per-psum-group critical path; ACT ring so its semaphore
                # wait can't block the SP ring's load prefetch
                nc.scalar.dma_start(y[t], ot[:])

    nc.compile()
    return nc


def get_nc(slices_per_core: int = SLICES_PER_CORE):
    if slices_per_core not in _NC_CACHE:
        _NC_CACHE[slices_per_core] = _build_nc(slices_per_core)
    return _NC_CACHE[slices_per_core]


def _pack_input(xs: np.ndarray) -> np.ndarray:
    """[S, H, W] fp32 -> [S/64, 128, 64*34] bf16 in the SBUF tile layout."""
    s = xs.shape[0]
    ntiles = s // TILE_SLICES
    # (t, s, u, g, wp) -> (t, wp, u, s, g)
    v = xs.reshape(ntiles, TILE_SLICES, H, G, 2).transpose(0, 4, 2, 1, 3)
    out = np.zeros((ntiles, 2, H, TILE_SLICES, GC), BF16_NP)
    out[:, :, :, :, 1 : 1 + G] = v.astype(BF16_NP)
    return np.ascontiguousarray(out.reshape(ntiles, 128, TILE_SLICES * GC))


def _unpack_output(yp: np.ndarray) -> np.ndarray:
    """[S/64, 128, 64*32] bf16 -> [S, H, W] fp32."""
    ntiles = yp.shape[0]
    # [(jp, i), (s, g)] -> [s, i, (g, jp)]
    v = yp.reshape(ntiles, 2, H, TILE_SLICES, G).transpose(0, 3, 2, 4, 1)
    return v.reshape(ntiles * TILE_SLICES, H, W).astype(np.float32)


def kernel(x: np.ndarray, kernel: np.ndarray, _trace: bool = False, **_tkw):
    x = np.asarray(x, np.float32)
    wmat = _build_wmat(kernel)
    b, c, h, w = x.shape
    xs = x.reshape(b * c, h, w)
    spc = (b * c) // N_CORES
    nc = get_nc(spc)
    in_maps = [
        {"x": _pack_input(xs[k * spc : (k + 1) * spc]), "w": wmat}
        for k in range(N_CORES)
    ]
    res = run_bass_kernel_spmd(
        nc, in_maps, list(range(N_CORES)), trace=_trace, **_tkw
    )
    out = np.concatenate(
        [_unpack_output(res.results[k]["y"]) for k in range(N_CORES)], axis=0
    )
    result = out.reshape(b, c, h, w)
    if _trace:
        return result, res
    return result


# revision 15
# speedup vs baseline: 1.4076x; 1.2186x over previous
"""Trainium2 Bass kernel for nn_Blur: 4x4 FIR depthwise blur with pad (2,1).

out[n,c,i,j] = sum_{a,b} K[a,b] * x[n,c, i+1-a, j+1-b]   (zero-padded)

Strategy (8 NeuronCores, pure data parallelism over the 8192 (n,c) slices,
bf16 I/O to halve HBM traffic — the 2e-2 gate leaves ~5x margin):
  - Each core processes 1024 slices of 64x64, 16 SBUF tiles of 64 slices.
  - W-parity packing: partition p = wp*64 + u (wp = w%2, u = h), free =
    (slice, gc) with gc = 1 + w//2 plus one zero pad column on each side.
    The 4 W-taps of an output column then span only THREE gc columns, so
    the whole 4x4 blur is 3 PSUM-accumulated matmuls (vs 4 for the
    member-packed layout): lhsT_sh[(wp,u),(jp,i)] = K[i+1-u, jp-wp+1-2sh].
  - bf16 matmuls run at 1 col/cycle; weights (1/16, 3/16, 9/16 scale) are
    exact in bf16, accumulation is fp32 in PSUM.
  - PSUM->SBUF copies cast fp32->bf16 and alternate DVE/ACT engines.
  - The host pre-permutes each core's shard into the exact SBUF tile
    layout, so every DMA descriptor is one contiguous run per partition.
"""

import sys
import types

import numpy as np
import ml_dtypes

import concourse.bacc as bacc
import concourse.mybir as mybir
from concourse.tile import TileContext
from concourse.bass_utils import run_bass_kernel_spmd

BF16_NP = ml_dtypes.bfloat16


def _install_ntff_hook():
    """Best-effort shim: this image's antenv lacks axon_hooks, which the
    trace=True path of run_bass_kernel_spmd imports. Harmless if unused."""
    if "antenv.axon_hooks" in sys.modules:
        return
    try:
        sys.path.insert(0, "/root/.axon_site")
        from trn_agent_boot.trn_boot import _ntff_profile_via_ctypes

        hook = _ntff_profile_via_ctypes("/opt/axon/libaxon_pjrt.so")
        mod = types.ModuleType("antenv.axon_hooks")
        mod.get_axon_ntff_profile_hook = lambda: hook
        mod.set_axon_ntff_profile_hook = lambda h: None
        sys.modules["antenv.axon_hooks"] = mod
    except Exception:
        pass


_install_ntff_hook()

N_CORES = 8
B, C, H, W = 32, 256, 64, 64
NSLICES = B * C                       # 8192
SLICES_PER_CORE = NSLICES // N_CORES  # 1024
TILE_SLICES = 64                      # slices per SBUF tile
G = W // 2                            # 32 w-parity column groups
GC = G + 2                            # + zero pad col on each side
QS = 16                               # slices per PSUM group (16*32 = 512;
                                      # walrus ISA check caps a matmul dst
                                      # at one 2KB PSUM bank)
F32 = mybir.dt.float32
BF16 = mybir.dt.bfloat16

_NC_CACHE = {}


def _build_wmat(K: np.ndarray) -> np.ndarray:
    """(4, 128, 128) bf16: per-shift stationary matrices (4th = warmup pad).

    lhsT_sh[(wp,u), (jp,i)] = K[a, b] with a = i+1-u, b = jp-wp+1-2*sh,
    for shifts sh in (-1, 0, +1); entries with a or b outside 0..3 are 0.
    """
    K = np.asarray(K, np.float32)
    wmat = np.zeros((4, 128, 128), np.float32)
    for si, sh in enumerate((-1, 0, 1)):
        for wp in range(2):
            for jp in range(2):
                b = jp - wp + 1 - 2 * sh
                if not 0 <= b <= 3:
                    continue
                T = np.zeros((H, H), np.float32)
                for i in range(H):
                    for u in range(max(0, i - 2), min(H, i + 2)):
                        T[u, i] = K[i + 1 - u, b]
                wmat[si, wp * 64 : wp * 64 + 64, jp * 64 : jp * 64 + 64] = T
    # pre-transpose to the SBUF layout [k, (b, m)] so the weight DMA is one
    # contiguous 1KB run per partition instead of 512 x 256B descriptors
    return np.ascontiguousarray(
        wmat.transpose(1, 0, 2).reshape(128, 4 * 128)
    ).astype(BF16_NP)


WARMUP_MMS = 7


def _build_nc(slices_per_core: int = SLICES_PER_CORE):
    ntiles = slices_per_core // TILE_SLICES
    nc = bacc.Bacc("TRN2", target_bir_lowering=False, debug=False)
    # DRAM layouts are the SBUF tile layouts (host pre-/post-permutes):
    #   x: [tile, p=(wp u), (s gc)] with gc zero-padded to 34 — bf16
    #   y: [tile, p=(jp i), (s g)] — bf16
    x = nc.dram_tensor(
        "x", [ntiles, 128, TILE_SLICES * GC], BF16, kind="ExternalInput"
    ).ap()
    wm = nc.dram_tensor("w", [128, 4 * 128], BF16, kind="ExternalInput").ap()
    y = nc.dram_tensor(
        "y", [ntiles, 128, TILE_SLICES * G], BF16, kind="ExternalOutput"
    ).ap()
    # sink for the PE warm-up matmuls (kept alive so DCE can't drop them)
    warm_out = nc.dram_tensor("warm", [128, 4], F32, kind="ExternalOutput").ap()

    with TileContext(nc) as tc:
        with (
            tc.tile_pool(name="wpool", bufs=1) as wpool,
            tc.tile_pool(name="xpool", bufs=8) as xpool,
            tc.tile_pool(name="opool", bufs=4) as opool,
            tc.tile_pool(name="pspool", bufs=8, space="PSUM") as pspool,
        ):
            # weights ride the ACT ring; the SP ring issues ONLY the
            # input-tile loads so prefetch is never head-of-line blocked
            # behind a store's semaphore wait (in-order sequencer)
            wsb = wpool.tile([128, 4, 128], BF16, name="wsb")
            nc.scalar.dma_start(wsb.rearrange("k b m -> k (b m)"), wm)

            # PE warm-up on an on-chip memset tile (no DMA dependency): the
            # HAM clock gate needs ~3us of continuous PE activity to open
            # (0.65/1.2 -> 2.4 GHz) before the real matmuls start.
            wz = wpool.tile([128, 512], BF16, name="wz")
            nc.gpsimd.memset(wz[:], 0)
            wscratch = wpool.tile([128, 4], F32, name="wscratch")
            wps = pspool.tile([128, QS, G], F32, name="wps", tag="ps")
            for r in range(WARMUP_MMS):
                nc.tensor.matmul(
                    wps[:],
                    wz[:, 0:128],
                    wz[:],
                    start=(r == 0),
                    stop=(r == WARMUP_MMS - 1),
                )

            for t in range(ntiles):
                xt = xpool.tile([128, TILE_SLICES, GC], BF16, name="xt")
                nc.sync.dma_start(xt[:], x[t])

                # one output tile per input tile; 2 psum-group copies fill it
                ot = opool.tile([128, TILE_SLICES, G], BF16, name="ot")
                for q in range(TILE_SLICES // QS):
                    ps = pspool.tile([128, QS, G], F32, name="ps")
                    for si in range(3):
                        nc.tensor.matmul(
                            ps[:],
                            wsb[:, si, :],
                            xt[:, QS * q : QS * q + QS, si : si + G],
                            start=(si == 0),
                            stop=(si == 2),
                        )
                    # alternate copy engine: DVE and ACT share the load
                    if q % 2 == 0:
                        nc.vector.tensor_copy(
                            ot[:, QS * q : QS * q + QS, :], ps[:]
                        )
                    else:
                        nc.scalar.copy(ot[:, QS * q : QS * q + QS, :], ps[:])
                    if t == 0 and q == 0:
                        # emitted here so its sequencer slot never blocks
                        # tile copies; frees the warmup psum slot
                        nc.vector.tensor_copy(wscratch[:], wps[:, 0, 0:4])

                # single whole-tile store: 16 store issues total instead of
                # 64, keeping sequencer descriptor-write cost off the
                # per-psum-group critical path; ACT ring so its semaphore
                # wait can't block the SP ring's load prefetch
                nc.scalar.dma_start(y[t], ot[:])

            # warm-up sink store last: keeps the ACT ring clear during the
            # steady state while still defeating DCE
            nc.scalar.dma_start(warm_out, wscratch[:])

    nc.compile()
    return nc


def get_nc(slices_per_core: int = SLICES_PER_CORE):
    if slices_per_core not in _NC_CACHE:
        _NC_CACHE[slices_per_core] = _build_nc(slices_per_core)
    return _NC_CACHE[slices_per_core]


def _pack_input(xs: np.ndarray) -> np.ndarray:
    """[S, H, W] fp32 -> [S/64, 128, 64*34] bf16 in the SBUF tile layout."""
    s = xs.shape[0]
    ntiles = s // TILE_SLICES
    # (t, s, u, g, wp) -> (t, wp, u, s, g)
    v = xs.reshape(ntiles, TILE_SLICES, H, G, 2).transpose(0, 4, 2, 1, 3)
    out = np.zeros((ntiles, 2, H, TILE_SLICES, GC), BF16_NP)
    out[:, :, :, :, 1 : 1 + G] = v.astype(BF16_NP)
    return np.ascontiguousarray(out.reshape(ntiles, 128, TILE_SLICES * GC))


def _unpack_output(yp: np.ndarray) -> np.ndarray:
    """[S/64, 128, 64*32] bf16 -> [S, H, W] fp32."""
    ntiles = yp.shape[0]
    # [(jp, i), (s, g)] -> [s, i, (g, jp)]
    v = yp.reshape(ntiles, 2, H, TILE_SLICES, G).transpose(0, 3, 2, 4, 1)
    return v.reshape(ntiles * TILE_SLICES, H, W).astype(np.float32)


def kernel(x: np.ndarray, kernel: np.ndarray, _trace: bool = False, **_tkw):
    x = np.asarray(x, np.float32)
    wmat = _build_wmat(kernel)
    b, c, h, w = x.shape
    xs = x.reshape(b * c, h, w)
    spc = (b * c) // N_CORES
    nc = get_nc(spc)
    in_maps = [
        {"x": _pack_input(xs[k * spc : (k + 1) * spc]), "w": wmat}
        for k in range(N_CORES)
    ]
    res = run_bass_kernel_spmd(
        nc, in_maps, list(range(N_CORES)), trace=_trace, **_tkw
    )
    out = np.concatenate(
        [_unpack_output(res.results[k]["y"]) for k in range(N_CORES)], axis=0
    )
    result = out.reshape(b, c, h, w)
    if _trace:
        return result, res
    return result
